# revision 1
# baseline (speedup 1.0000x reference)
"""DarkChannel Trainium2 kernel.

Computes, per image: channel-min over C=3, then 15x15 sliding-window min
with reflect padding (== clamped-window min, since reflected indices always
fall inside the clamped window), over [B,3,512,512] f32 -> [B,1,512,512].

Sharding: pure data parallel, batch 16 -> 2 images on each of 8 cores.

Algorithm per image (per core):
  1. DMA the 3 channel planes into SBUF, rows on partitions
     (4 row-tiles of 128 rows, kept in one [128,4,512] free layout).
  2. channel-min (2 tensor_tensor min passes) into a 526-wide buffer whose
     7-col borders are preset to +BIG (clamped-window erosion == erosion
     with +inf padding).
  3. Horizontal 15-tap min via log-doubling shifts (1,2,4,7): 4 TT passes.
  4. Transpose 512x512 via TensorE (16 128x128 blocks -> PSUM), ScalarE
     copies PSUM->SBUF into another padded buffer.
  5. "Vertical" pass = same 4 shift passes on the transposed image.
  6. Transpose back via TensorE, ScalarE copy, DMA out.

All min passes run on VectorE (fp32 tensor_tensor, 1 elem/cycle/lane);
this walrus build rejects Pool-engine TensorTensor and CCE-min DMAs, so
DVE is the only streaming-min engine and sets the steady-state floor.
"""

import numpy as np

import concourse.bacc as bacc
import concourse.mybir as mybir
from concourse.tile import TileContext
from concourse.masks import make_identity
from concourse.bass_utils import run_bass_kernel_spmd

F32 = mybir.dt.float32
MIN = mybir.AluOpType.min

P = 128          # SBUF partitions
H = W = 512
NT = 4           # row-tiles (128 rows each) per image
PAD = 7
PW = W + 2 * PAD  # 526
BIG = 1.0e30
B_PER_CORE = 2
N_CORES = 8


def _build(repeat=1, n_images=B_PER_CORE, ngrp=2, vgrp=None, merge_chains=False,
           split_load=True, split_store=True, split_s7=False, xin_bufs=2,
           work_bufs=4, himg_bufs=2, out_bufs=2, psum_bufs=4):
    """Build + compile the Bacc program. Returns nc.

    ngrp: row-groups for the chan-min + horizontal stage (finer = earlier
    start after partial loads). vgrp: col-groups for the vertical stage
    (coarser = fewer DVE instructions); defaults to ngrp."""
    tpg = NT // ngrp
    if vgrp is None:
        vgrp = ngrp
    tpv = NT // vgrp
    nc = bacc.Bacc("TRN2", target_bir_lowering=False, debug=False)
    x = nc.declare_dram_parameter("x", [n_images, 3, H, W], F32, isOutput=False)
    y = nc.declare_dram_parameter("y", [n_images, 1, H, W], F32, isOutput=True)

    with TileContext(nc) as tc:
        with (
            tc.tile_pool(name="consts", bufs=1) as consts,
            tc.tile_pool(name="xin", bufs=xin_bufs) as xin_pool,
            tc.tile_pool(name="tmp", bufs=2) as tmp_pool,
            tc.tile_pool(name="work", bufs=work_bufs) as work_pool,
            tc.tile_pool(name="himg", bufs=himg_bufs) as h_pool,
            tc.tile_pool(name="outp", bufs=out_bufs) as out_pool,
            tc.tile_pool(name="ps", bufs=psum_bufs, space="PSUM") as psum_pool,
        ):
            ident = consts.tile([P, P], F32)
            make_identity(nc, ident)

            def _chain(buf, nt):
                cur, wid = buf, PW
                for s in (1, 2, 4, 7):
                    nw = wid - s
                    if s != 7:
                        nxt = work_pool.tile([P, nt, PW], F32, tag="work")
                        nc.vector.tensor_tensor(
                            out=nxt[:, :, 0:nw], in0=cur[:, :, 0:nw],
                            in1=cur[:, :, s:s + nw], op=MIN,
                        )
                    else:
                        nxt = h_pool.tile([P, nt, W], F32, tag="himg")
                        if split_s7:
                            for jj in range(nt):
                                nc.vector.tensor_tensor(
                                    out=nxt[:, jj], in0=cur[:, jj, 0:W],
                                    in1=cur[:, jj, s:s + W], op=MIN,
                                )
                        else:
                            nc.vector.tensor_tensor(
                                out=nxt[:, :, 0:nw], in0=cur[:, :, 0:nw],
                                in1=cur[:, :, s:s + nw], op=MIN,
                            )
                    cur, wid = nxt, nw
                return cur

            for _rep in range(repeat):
                for b in range(n_images):
                    # ---- load: 3 channel planes, rows->partitions ----
                    X = xin_pool.tile([P, 3, NT, W], F32, tag="xin")
                    xr = x[b].rearrange("c (i p) w -> p c i w", p=P)
                    if split_load:
                        # one DMA per (half, channel), first half's channels
                        # first -> the first chan-min can start after ~1.5MB
                        for hlf in range(2):
                            i0, i1 = hlf * 2, hlf * 2 + 2
                            for c in range(3):
                                nc.sync.dma_start(
                                    out=X[:, c, i0:i1], in_=xr[:, c, i0:i1]
                                )
                    else:
                        for c in range(3):
                            nc.sync.dma_start(out=X[:, c], in_=xr[:, c])

                    # ---- channel-min (split for early start) into ONE
                    # whole-image padded buffer, then merged h-chain ----
                    if merge_chains:
                        Pb = work_pool.tile([P, NT, PW], F32, tag="work")
                        nc.vector.memset(Pb[:, :, 0:PAD], BIG)
                        nc.vector.memset(Pb[:, :, PAD + W:PW], BIG)
                        for g in range(ngrp):
                            t0, t1 = g * tpg, (g + 1) * tpg
                            T = tmp_pool.tile([P, tpg, W], F32, tag="tmp")
                            nc.vector.tensor_tensor(
                                out=T[:], in0=X[:, 0, t0:t1],
                                in1=X[:, 1, t0:t1], op=MIN,
                            )
                            nc.vector.tensor_tensor(
                                out=Pb[:, t0:t1, PAD:PAD + W], in0=T[:],
                                in1=X[:, 2, t0:t1], op=MIN,
                            )
                        hmins = [_chain(Pb, NT)]
                        htpg = NT
                    else:
                        hmins = []
                        for g in range(ngrp):
                            t0, t1 = g * tpg, (g + 1) * tpg
                            T = tmp_pool.tile([P, tpg, W], F32, tag="tmp")
                            nc.vector.tensor_tensor(
                                out=T[:], in0=X[:, 0, t0:t1],
                                in1=X[:, 1, t0:t1], op=MIN,
                            )
                            Pb = work_pool.tile([P, tpg, PW], F32, tag="work")
                            nc.vector.memset(Pb[:, :, 0:PAD], BIG)
                            nc.vector.memset(Pb[:, :, PAD + W:PW], BIG)
                            nc.vector.tensor_tensor(
                                out=Pb[:, :, PAD:PAD + W], in0=T[:],
                                in1=X[:, 2, t0:t1], op=MIN,
                            )
                            hmins.append(_chain(Pb, tpg))
                        htpg = tpg

                    # ---- transpose HMin -> padded vertical buffers ----
                    vbufs = []
                    for g in range(vgrp):  # g indexes col-groups now
                        Vb = work_pool.tile([P, tpv, PW], F32, tag="work")
                        nc.vector.memset(Vb[:, :, 0:PAD], BIG)
                        nc.vector.memset(Vb[:, :, PAD + W:PW], BIG)
                        for jj in range(tpv):
                            j = g * tpv + jj  # absolute col-tile
                            TP = psum_pool.tile([P, W], F32, tag="tp")
                            for i in range(NT):  # absolute row-tile
                                hg = hmins[i // htpg]
                                nc.tensor.transpose(
                                    TP[:, i * P:(i + 1) * P],
                                    hg[:, i % htpg, j * P:(j + 1) * P],
                                    ident,
                                )
                            nc.scalar.copy(out=Vb[:, jj, PAD:PAD + W], in_=TP[:])
                        vbufs.append(Vb)

                    # ---- vertical erosion (free axis = rows now) ----
                    vmins = []
                    for g in range(vgrp):
                        vmins.append(_chain(vbufs[g], tpv))  # [P=cols, tpv, W=rows]

                    # ---- transpose back + store ----
                    yr = y[b, 0].rearrange("(i p) w -> p i w", p=P)
                    OUT = out_pool.tile([P, NT, W], F32, tag="outp")
                    for i in range(NT):
                        TO = psum_pool.tile([P, W], F32, tag="to")
                        for j in range(NT):
                            vg = vmins[j // tpv]
                            nc.tensor.transpose(
                                TO[:, j * P:(j + 1) * P],
                                vg[:, j % tpv, i * P:(i + 1) * P],
                                ident,
                            )
                        nc.scalar.copy(out=OUT[:, i], in_=TO[:])
                        if split_store:
                            nc.sync.dma_start(out=yr[:, i], in_=OUT[:, i])
                    if not split_store:
                        nc.sync.dma_start(out=yr, in_=OUT[:])
    nc.compile()
    return nc


_CACHE = {}


def _get_nc(**kw):
    key = tuple(sorted(kw.items()))
    if key not in _CACHE:
        _CACHE[key] = _build(**kw)
    return _CACHE[key]


def kernel(x: np.ndarray) -> np.ndarray:
    """Full-input entry point: x [16,3,512,512] f32 -> [16,1,512,512] f32."""
    x = np.ascontiguousarray(x, dtype=np.float32)
    B = x.shape[0]
    assert B == N_CORES * B_PER_CORE, x.shape
    nc = _get_nc()
    in_maps = [
        {"x": x[c * B_PER_CORE:(c + 1) * B_PER_CORE]} for c in range(N_CORES)
    ]
    res = run_bass_kernel_spmd(nc, in_maps, core_ids=list(range(N_CORES)))
    out = np.concatenate([res.results[c]["y"] for c in range(N_CORES)], axis=0)
    return out.astype(np.float32, copy=False)



# revision 2
# speedup vs baseline: 2.5756x; 2.5756x over previous
"""DarkChannel Trainium2 kernel — bf16 shift-chain erosion.

Per image [3,512,512]: channel-min over C=3, then 15x15 sliding-window min
with reflect padding (== clamped-window min == +BIG-pad erosion), computed
as two separable 15-tap min chains (shifts 1,2,4,7) with a TensorE
transpose between them.

vs. the f32 baseline: all device compute and I/O in bf16 (host casts the
input f32->bf16 and upcasts the output back; mins of bf16 values are exact,
so total error is the input rounding, ~2^-9 rel << the 2e-2 gate).
bf16 TensorTensor runs in the DVE 2x perf mode (measured ~2.1x faster than
f32 per pass on HW), and DMA bytes halve.

Sharding: pure data parallel, batch 16 -> 2 images on each of 8 cores.
Input is host-permuted to [b, p, c, i, w] so every DMA descriptor is a long
contiguous run per partition; output returns [b, p, i, w] bf16.
"""

import numpy as np
import ml_dtypes

import concourse.bacc as bacc
import concourse.mybir as mybir
from concourse.tile import TileContext
from concourse.masks import make_identity
from concourse.bass_utils import run_bass_kernel_spmd

F32 = mybir.dt.float32
BF16 = mybir.dt.bfloat16
MIN = mybir.AluOpType.min

P = 128          # SBUF partitions
H = W = 512
NT = 4           # row-tiles (128 rows each) per image
PAD = 7
PW = W + 2 * PAD  # 526
BIG = 1.0e30
B_PER_CORE = 2
N_CORES = 8


def _build(repeat=1, n_images=B_PER_CORE, ngrp=2, vgrp=None,
           split_load=True, split_store=True, xin_bufs=2,
           work_bufs=4, himg_bufs=2, out_bufs=2, psum_bufs=4,
           merge_imgs=True):
    """ngrp: row-groups for the chan-min + horizontal stage. vgrp:
    col-groups for the vertical stage; defaults to ngrp.
    merge_imgs: process both images as one 8-row-tile virtual image
    (fewer, larger DVE ops)."""
    if merge_imgs:
        return _build_merged(repeat=repeat, n_images=n_images, ngrp=ngrp,
                             vgrp=vgrp, xin_bufs=xin_bufs,
                             work_bufs=work_bufs, himg_bufs=himg_bufs,
                             out_bufs=out_bufs, psum_bufs=psum_bufs)
    tpg = NT // ngrp
    if vgrp is None:
        vgrp = ngrp
    tpv = NT // vgrp
    nc = bacc.Bacc("TRN2", target_bir_lowering=False, debug=False)
    x = nc.declare_dram_parameter("x", [n_images, P, 3, NT, W], BF16,
                                  isOutput=False)
    y = nc.declare_dram_parameter("y", [n_images, P, NT, W], BF16,
                                  isOutput=True)

    with TileContext(nc) as tc:
        with (
            tc.tile_pool(name="consts", bufs=1) as consts,
            tc.tile_pool(name="xin", bufs=xin_bufs) as xin_pool,
            tc.tile_pool(name="tmp", bufs=2) as tmp_pool,
            tc.tile_pool(name="work", bufs=work_bufs) as work_pool,
            tc.tile_pool(name="himg", bufs=himg_bufs) as h_pool,
            tc.tile_pool(name="outp", bufs=out_bufs) as out_pool,
            tc.tile_pool(name="ps", bufs=psum_bufs, space="PSUM") as psum_pool,
        ):
            ident = consts.tile([P, P], BF16)
            make_identity(nc, ident)

            def _chain(buf, nt):
                cur, wid = buf, PW
                for s in (1, 2, 4, 7):
                    nw = wid - s
                    if s != 7:
                        nxt = work_pool.tile([P, nt, PW], BF16, tag="work")
                        nc.vector.tensor_tensor(
                            out=nxt[:, :, 0:nw], in0=cur[:, :, 0:nw],
                            in1=cur[:, :, s:s + nw], op=MIN,
                        )
                    else:
                        nxt = h_pool.tile([P, nt, W], BF16, tag="himg")
                        nc.vector.tensor_tensor(
                            out=nxt[:, :, 0:nw], in0=cur[:, :, 0:nw],
                            in1=cur[:, :, s:s + nw], op=MIN,
                        )
                    cur, wid = nxt, nw
                return cur

            for _rep in range(repeat):
                for b in range(n_images):
                    # ---- load: [P, 3, NT, W] bf16, contiguous/partition ----
                    X = xin_pool.tile([P, 3, NT, W], BF16, tag="xin")
                    if split_load:
                        for c in range(3):
                            nc.sync.dma_start(out=X[:, c], in_=x[b, :, c])
                    else:
                        nc.sync.dma_start(out=X, in_=x[b])

                    # ---- channel-min + horizontal chains (per row-group) --
                    hmins = []
                    for g in range(ngrp):
                        t0, t1 = g * tpg, (g + 1) * tpg
                        T = tmp_pool.tile([P, tpg, W], BF16, tag="tmp")
                        nc.vector.tensor_tensor(
                            out=T[:], in0=X[:, 0, t0:t1],
                            in1=X[:, 1, t0:t1], op=MIN,
                        )
                        Pb = work_pool.tile([P, tpg, PW], BF16, tag="work")
                        nc.vector.memset(Pb[:, :, 0:PAD], BIG)
                        nc.vector.memset(Pb[:, :, PAD + W:PW], BIG)
                        nc.vector.tensor_tensor(
                            out=Pb[:, :, PAD:PAD + W], in0=T[:],
                            in1=X[:, 2, t0:t1], op=MIN,
                        )
                        hmins.append(_chain(Pb, tpg))
                    htpg = tpg

                    # ---- transpose HMin -> padded vertical buffers ----
                    vbufs = []
                    for g in range(vgrp):  # g indexes col-groups now
                        Vb = work_pool.tile([P, tpv, PW], BF16, tag="work")
                        nc.vector.memset(Vb[:, :, 0:PAD], BIG)
                        nc.vector.memset(Vb[:, :, PAD + W:PW], BIG)
                        for jj in range(tpv):
                            j = g * tpv + jj  # absolute col-tile
                            TP = psum_pool.tile([P, W], BF16, tag="tp")
                            for i in range(NT):  # absolute row-tile
                                hg = hmins[i // htpg]
                                nc.tensor.transpose(
                                    TP[:, i * P:(i + 1) * P],
                                    hg[:, i % htpg, j * P:(j + 1) * P],
                                    ident,
                                )
                            nc.scalar.copy(out=Vb[:, jj, PAD:PAD + W], in_=TP[:])
                        vbufs.append(Vb)

                    # ---- vertical erosion (free axis = rows now) ----
                    vmins = []
                    for g in range(vgrp):
                        vmins.append(_chain(vbufs[g], tpv))

                    # ---- transpose back + store ----
                    OUT = out_pool.tile([P, NT, W], BF16, tag="outp")
                    for i in range(NT):
                        TO = psum_pool.tile([P, W], BF16, tag="to")
                        for j in range(NT):
                            vg = vmins[j // tpv]
                            nc.tensor.transpose(
                                TO[:, j * P:(j + 1) * P],
                                vg[:, j % tpv, i * P:(i + 1) * P],
                                ident,
                            )
                        nc.scalar.copy(out=OUT[:, i], in_=TO[:])
                        if split_store:
                            nc.sync.dma_start(out=y[b, :, i], in_=OUT[:, i])
                    if not split_store:
                        nc.sync.dma_start(out=y[b], in_=OUT[:])
    nc.compile()
    return nc


def _build_merged(repeat=1, n_images=B_PER_CORE, ngrp=1, vgrp=None,
                  xin_bufs=2, work_bufs=4, himg_bufs=2, out_bufs=2,
                  psum_bufs=4):
    """Both images form one virtual image of NTT = n_images*NT row-tiles;
    every DVE pass covers both images (fewer, larger instructions)."""
    NTT = n_images * NT
    tpg = NTT // ngrp
    if vgrp is None:
        vgrp = ngrp
    tpv = NTT // vgrp
    nc = bacc.Bacc("TRN2", target_bir_lowering=False, debug=False)
    x = nc.declare_dram_parameter("x", [n_images, P, 3, NT, W], BF16,
                                  isOutput=False)
    y = nc.declare_dram_parameter("y", [n_images, P, NT, W], BF16,
                                  isOutput=True)

    with TileContext(nc) as tc:
        with (
            tc.tile_pool(name="consts", bufs=1) as consts,
            tc.tile_pool(name="xin", bufs=xin_bufs) as xin_pool,
            tc.tile_pool(name="tmp", bufs=2) as tmp_pool,
            tc.tile_pool(name="work", bufs=work_bufs) as work_pool,
            tc.tile_pool(name="himg", bufs=himg_bufs) as h_pool,
            tc.tile_pool(name="outp", bufs=out_bufs) as out_pool,
            tc.tile_pool(name="ps", bufs=psum_bufs, space="PSUM") as psum_pool,
        ):
            ident = consts.tile([P, P], BF16)
            make_identity(nc, ident)

            def _chain(buf, nt):
                cur, wid = buf, PW
                for s in (1, 2, 4, 7):
                    nw = wid - s
                    pool = work_pool if s != 7 else h_pool
                    tagn = "work" if s != 7 else "himg"
                    outw = PW if s != 7 else W
                    nxt = pool.tile([P, nt, outw], BF16, tag=tagn)
                    nc.vector.tensor_tensor(
                        out=nxt[:, :, 0:nw], in0=cur[:, :, 0:nw],
                        in1=cur[:, :, s:s + nw], op=MIN,
                    )
                    cur, wid = nxt, nw
                return cur

            for _rep in range(repeat):
                # ---- load both images: [P, 3, NTT, W] ----
                X = xin_pool.tile([P, 3, NTT, W], BF16, tag="xin")
                for b in range(n_images):
                    for c in range(3):
                        nc.sync.dma_start(out=X[:, c, b * NT:(b + 1) * NT],
                                          in_=x[b, :, c])

                # ---- channel-min + horizontal chains ----
                hmins = []
                for g in range(ngrp):
                    t0, t1 = g * tpg, (g + 1) * tpg
                    T = tmp_pool.tile([P, tpg, W], BF16, tag="tmp")
                    nc.vector.tensor_tensor(
                        out=T[:], in0=X[:, 0, t0:t1], in1=X[:, 1, t0:t1],
                        op=MIN,
                    )
                    Pb = work_pool.tile([P, tpg, PW], BF16, tag="work")
                    nc.vector.memset(Pb[:, :, 0:PAD], BIG)
                    nc.vector.memset(Pb[:, :, PAD + W:PW], BIG)
                    nc.vector.tensor_tensor(
                        out=Pb[:, :, PAD:PAD + W], in0=T[:],
                        in1=X[:, 2, t0:t1], op=MIN,
                    )
                    hmins.append(_chain(Pb, tpg))
                htpg = tpg

                # ---- transpose -> padded vertical buffers ----
                # vertical virtual tiles: v-tile (b, j) at index b*NT + j
                vbufs = []
                for g in range(vgrp):
                    Vb = work_pool.tile([P, tpv, PW], BF16, tag="work")
                    nc.vector.memset(Vb[:, :, 0:PAD], BIG)
                    nc.vector.memset(Vb[:, :, PAD + W:PW], BIG)
                    for jj in range(tpv):
                        vt = g * tpv + jj
                        b, j = vt // NT, vt % NT
                        TP = psum_pool.tile([P, W], BF16, tag="tp")
                        for i in range(NT):  # row-tile within image b
                            it = b * NT + i
                            hg = hmins[it // htpg]
                            nc.tensor.transpose(
                                TP[:, i * P:(i + 1) * P],
                                hg[:, it % htpg, j * P:(j + 1) * P],
                                ident,
                            )
                        nc.scalar.copy(out=Vb[:, jj, PAD:PAD + W], in_=TP[:])
                    vbufs.append(Vb)

                # ---- vertical erosion ----
                vmins = []
                for g in range(vgrp):
                    vmins.append(_chain(vbufs[g], tpv))

                # ---- transpose back + store ----
                for b in range(n_images):
                    OUT = out_pool.tile([P, NT, W], BF16, tag="outp")
                    for i in range(NT):
                        TO = psum_pool.tile([P, W], BF16, tag="to")
                        for j in range(NT):
                            vt = b * NT + j
                            vg = vmins[vt // tpv]
                            nc.tensor.transpose(
                                TO[:, j * P:(j + 1) * P],
                                vg[:, vt % tpv, i * P:(i + 1) * P],
                                ident,
                            )
                        nc.scalar.copy(out=OUT[:, i], in_=TO[:])
                        nc.sync.dma_start(out=y[b, :, i], in_=OUT[:, i])
    nc.compile()
    return nc


_CACHE = {}


def _get_nc(**kw):
    key = tuple(sorted(kw.items()))
    if key not in _CACHE:
        _CACHE[key] = _build(**kw)
    return _CACHE[key]


def _prep_input(x: np.ndarray) -> np.ndarray:
    x = np.ascontiguousarray(x, dtype=np.float32)
    b = x.shape[0]
    # [b, c, (i p), w] -> [b, p, c, i, w], bf16
    xr = x.reshape(b, 3, NT, P, W).transpose(0, 3, 1, 2, 4)
    return np.ascontiguousarray(xr).astype(ml_dtypes.bfloat16)


def kernel(x: np.ndarray) -> np.ndarray:
    """Full-input entry point: x [16,3,512,512] f32 -> [16,1,512,512] f32."""
    B = x.shape[0]
    assert B == N_CORES * B_PER_CORE, x.shape
    xb = _prep_input(x)
    nc = _get_nc()
    in_maps = [
        {"x": xb[c * B_PER_CORE:(c + 1) * B_PER_CORE]} for c in range(N_CORES)
    ]
    res = run_bass_kernel_spmd(nc, in_maps, core_ids=list(range(N_CORES)))
    yb = np.concatenate([res.results[c]["y"] for c in range(N_CORES)], axis=0)
    # [b, p, i, w] -> [b, 1, (i p), w]
    out = yb.astype(np.float32).transpose(0, 2, 1, 3).reshape(B, 1, H, W)
    return np.ascontiguousarray(out)
